# revision 20
# baseline (speedup 1.0000x reference)
"""ALSH ConvNet Trainium2 kernel: 8-core data-parallel over batch.

Per core (4 images): conv1(3->5,3x3)+mask1+relu -> conv2(5->5,3x3) in
TRANSPOSED orientation (h1 tile is the stationary matmul operand, banded-W2
constant streams) so conv2's output lands as [w, (co,h)] — exactly the
linear layer's rhs layout. No transpose, no DRAM round trip.

conv1 runs in f32r: x is DMA'd once and bitcast to f32r (free-dim >= 256
keeps f32r at full rate), no DVE cast pass. Layer-1 query hash (mask1) is
computed on the host and shipped as a tiny per-image input; layer-2 query
patch sums are accumulated on device (ACT accum_out folded into conv1
drains) and returned; the host finishes the hash and applies mask2 + bias
to the returned output (mask commutes through relu and the linear).
"""
import numpy as np
import concourse.bass as bass
from concourse import bacc
import concourse.tile as tile
import concourse.mybir as mybir
from concourse.bass_utils import run_bass_kernel_spmd

f32 = mybir.dt.float32
f16 = mybir.dt.float16
AF = mybir.ActivationFunctionType
ALU = mybir.AluOpType
AX = mybir.AxisListType

R = 0.1
U = 0.99
BC = 4            # images per core
H, WX = 260, 1004
H1, W1W = 258, 1002
H2, W2W = 256, 1000
NT = 12           # row tiles: 11 overlapped full tiles + last
# conv1: x rows [23t,23t+27) -> h1 rows [23t,23t+25), K=81, M=125
# conv1 last (t=11): x rows [253,260) -> h1 rows [253,258), K=21, M=25
# conv2T tile t<11: h2 rows [23t,23t+23) x w-chunk: lhsT=h1[125, 128w],
#   rhs=w2bt[dw][125,115], out psum [128w, 115=(co,hr)]
# conv2T t=11: h1L[25, w], rhs [25, 15]
W1CH = [(0, 512), (512, 490)]
XCH = [(0, 3), (3, 4), (7, 4)]  # x DMA chunks (tile ranges)
NCCH = [(0, 512), (512, 512), (1024, 256)]
LKS = [(k * 128, 128) for k in range(7)] + [(896, 104)]
# h2T column layout (scrambled; host unscrambles): t<11: t*115 + co*23 + hr
# t=11: 1265 + co*3 + hr


def _filter_hash(W, a, b):
    Cout = W.shape[0]
    wf = W.reshape(Cout, -1).astype(np.float32)
    norms = np.sqrt((wf * wf).sum(1))
    ws = wf * np.float32(U / norms.max())
    n2 = (ws * ws).sum(1)
    powers = np.stack([n2, n2**2, n2**4, n2**8, n2**16], axis=1)
    Pw = np.concatenate([ws, powers], axis=1).astype(np.float32)
    return np.mod(np.floor(
        (Pw @ a.astype(np.float32) + np.float32(b)) / np.float32(R)
    ).astype(np.int64), 2).astype(np.int64)


def _qhash(q, a, b):
    # q: [B, d] raw patch sums (scale cancels in normalization)
    qn = q / np.maximum(np.linalg.norm(q, axis=1, keepdims=True), 1e-12)
    v = qn @ a[:q.shape[1]].astype(np.float64) \
        + 0.5 * a[q.shape[1]:].astype(np.float64).sum() + float(b)
    return np.mod(np.floor(v / R).astype(np.int64), 2)


def _build_lhsT(W, Cin, KH, THo):
    # [3(dj), Cin*KH, 5*THo]; lhsT[dj][(ci*KH+dh),(co*THo+u)] = W[co,ci,dh-u,dj]
    L = np.zeros((3, Cin * KH, 5 * THo), np.float32)
    for dj in range(3):
        for co in range(5):
            for ci in range(Cin):
                for u in range(THo):
                    for di in range(3):
                        L[dj, ci * KH + u + di, co * THo + u] = W[co, ci, di, dj]
    return L


def _build_w2bt(W2, HH, HO):
    # [3(dw), 5*HH, 5*HO]; rhs[dw][(ci*HH+u),(co*HO+hr)] = W2[co,ci,u-hr,dw]
    B = np.zeros((3, 5 * HH, 5 * HO), np.float32)
    for dw in range(3):
        for co in range(5):
            for ci in range(5):
                for hr in range(HO):
                    for dh in range(3):
                        if hr + dh < HH:
                            B[dw, ci * HH + hr + dh, co * HO + hr] = \
                                W2[co, ci, dh, dw]
    return B


def _ind_blk(nvalid, KH, Cin, t, win):
    # [Cin*KH, Cin*3]; ind[(ci*KH+dh),(ci*3+i)]=1 iff dh<nvalid and
    # 23t+dh in [i, i+win)
    out = np.zeros((Cin * KH, Cin * 3), np.float32)
    for ci in range(Cin):
        for dh in range(KH):
            hg = 23 * t + dh
            for i in range(3):
                if dh < nvalid and i <= hg < i + win:
                    out[ci * KH + dh, ci * 3 + i] = 1.0
    return out


def _consts_full(W1n, W2n, Wln):
    # per-family consts packed side-by-side in the free dim: one DMA each
    c = {}
    c["l1"] = np.concatenate(
        list(_build_lhsT(W1n, 3, 27, 25)), axis=1).astype(np.float16)
    c["l1L"] = np.concatenate(
        list(_build_lhsT(W1n, 3, 7, 5)), axis=1).astype(np.float16)
    c["w2bt"] = np.concatenate(
        list(_build_w2bt(W2n, 25, 23)), axis=1).astype(np.float16)
    c["w2btL"] = np.concatenate(
        list(_build_w2bt(W2n, 5, 3)), axis=1).astype(np.float16)
    WlT = Wln.T.astype(np.float32)             # [1000, 10]
    wltf = np.zeros((128, 80), np.float32)
    for k, (k0, K) in enumerate(LKS):
        wltf[:K, k * 10:k * 10 + 10] = WlT[k0:k0 + K]
    c["wlt"] = wltf.astype(np.float16)

    ind_h = np.zeros((125, 165), np.float32)
    for t in range(11):
        ind_h[:, t * 15:t * 15 + 15] = _ind_blk(23, 25, 5, t, 256)
    c["ind_h"] = ind_h
    c["ind_hL"] = _ind_blk(5, 5, 5, 11, 256)   # [25, 15]
    return c


_CDTYPES = {"l1": f16, "l1L": f16, "w2bt": f16, "w2btL": f16, "wlt": f16,
            "ind_h": f32, "ind_hL": f32, "m1": f32, "m1L": f32}


def _build_nc(cshapes):
    nc = bacc.Bacc("TRN2", target_bir_lowering=False)
    xP = nc.declare_dram_parameter("x", [BC, 912, WX], f16, isOutput=False)
    outP = nc.declare_dram_parameter("out", [BC, 10, 1280], f32, isOutput=True)
    q2sP = nc.declare_dram_parameter("q2s", [BC, 15, 3], f32, isOutput=True)
    cP = {}
    for k, (shp, dt_) in cshapes.items():
        cP[k] = nc.declare_dram_parameter(k, list(shp), dt_, isOutput=False)

    with tile.TileContext(nc) as tc:
        with tc.tile_pool(name="consts", bufs=1) as cpool, \
             tc.tile_pool(name="xt", bufs=5) as xtp, \
             tc.tile_pool(name="xl", bufs=2) as xlp, \
             tc.tile_pool(name="wp", bufs=4) as wp, \
             tc.tile_pool(name="rpp", bufs=13) as rpp, \
             tc.tile_pool(name="h1p", bufs=1) as h1p, \
             tc.tile_pool(name="h2tp", bufs=1) as h2tp, \
             tc.tile_pool(name="outp", bufs=2) as outp, \
             tc.tile_pool(name="cps", bufs=2, space="PSUM") as cps, \
             tc.tile_pool(name="c2ps", bufs=3, space="PSUM") as c2ps, \
             tc.tile_pool(name="qps", bufs=1, space="PSUM") as qps, \
             tc.tile_pool(name="lps", bufs=2, space="PSUM") as lps:

            # image-0 x DMAs go first so conv1 can start ASAP; consts
            # stream in behind them on the HWDGE.
            def load_x(b):
                xcs = []
                for (t0, ntl) in XCH:
                    xc = xtp.tile([81, 4 * WX], f16, tag="xc")
                    nc.sync.dma_start(
                        xc[:, 0:ntl * WX].rearrange("p (k w) -> p k w",
                                                    k=ntl),
                        xP[b, t0 * 81:(t0 + ntl) * 81, :].rearrange(
                            "(k p) w -> p k w", k=ntl))
                    xcs.append(xc)
                xl = xlp.tile([21, WX], f16, tag="xl")
                nc.sync.dma_start(xl[:], xP[b, 891:912, :])
                return xcs, xl

            # consts: DMA -> SBUF, then DVE copy so matmul operands are
            # engine-produced (avoids extra DMA-queue waits on matmuls)
            cs = {}

            def load_consts(names):
                for k in names:
                    shp, dt_ = cshapes[k]
                    shp2 = list(shp) if len(shp) == 2 else list(shp[1:])
                    n3 = shp[0] if len(shp) == 3 else None
                    for j in range(n3 or 1):
                        nm = f"{k}{j}" if n3 else k
                        tmp = cpool.tile(shp2, dt_, tag=nm + "_d")
                        nc.sync.dma_start(tmp[:], cP[k][j] if n3 else cP[k][:])
                        t_ = cpool.tile(shp2, dt_, tag=nm)
                        nc.vector.tensor_copy(t_[:], tmp[:])
                        cs[nm] = t_

            load_consts(["l1"])            # needed by the very first matmul
            x0 = load_x(0)
            load_consts(["m1", "l1L", "m1L", "ind_h", "ind_hL",
                         "w2bt", "w2btL", "wlt"])
            xnext = x0
            for b in range(BC):
                xcs, xl = xnext

                # ---- conv1 (f32r) + q2 patch sums ----
                h1t = h1p.tile([125, 11 * W1W], f16, tag="h1t")
                h1L = h1p.tile([25, W1W], f16, tag="h1L")
                psq_args = []
                for t in range(NT):
                    MT = 125 if t < 11 else 25
                    if t < 11:
                        ch = next(i for i, (t0, n_) in enumerate(XCH)
                                  if t0 <= t < t0 + n_)
                        rhsrc = xcs[ch]
                        base = (t - XCH[ch][0]) * WX
                        lt, lw = "l1", 125
                        hsl = h1t[:, t * W1W:(t + 1) * W1W]
                    else:
                        rhsrc = xl
                        base = 0
                        lt, lw = "l1L", 25
                        hsl = h1L[:]
                    accs = []
                    m1ap = (cs["m1"][0:MT, b:b + 1] if t < 11
                            else cs["m1L"][:, b:b + 1])
                    for wi, (w0, N) in enumerate(W1CH):
                        ps = cps.tile([125, 512], f32, tag="cps")
                        for dj in range(3):
                            nc.tensor.matmul(
                                ps[0:MT, 0:N],
                                cs[lt][:, dj * lw:(dj + 1) * lw],
                                rhsrc[:, base + w0 + dj:base + w0 + dj + N],
                                start=(dj == 0), stop=(dj == 2))
                        dst = (h1t[:, t * W1W + w0: t * W1W + w0 + N]
                               if t < 11 else h1L[:, w0:w0 + N])
                        acc = wp.tile([MT, 1], f32, tag=f"acc{wi}")
                        if wi == 0:
                            # chunk A drains on ACT (relu*mask + row sums)
                            nc.scalar.activation(dst, ps[0:MT, 0:N], AF.Relu,
                                                 scale=m1ap, accum_out=acc[:])
                        else:
                            # chunk B drains on DVE: (psum max 0) * mask
                            nc.vector.tensor_scalar(
                                dst, ps[0:MT, 0:N], 0.0, m1ap,
                                op0=ALU.max, op1=ALU.mult, accum_out=acc[:])
                        accs.append(acc)
                    S2 = wp.tile([MT, 1], f32, tag="q2S")
                    nc.vector.tensor_add(S2[:], accs[0][:], accs[1][:])
                    t1 = wp.tile([MT, 1], f32, tag="q2t1")
                    nc.vector.tensor_sub(t1[:], S2[:], hsl[:, 1001:1002])
                    t2 = wp.tile([MT, 1], f32, tag="q2t2")
                    nc.vector.tensor_sub(t2[:], S2[:], hsl[:, 0:1])
                    rp = rpp.tile([MT, 3], f32, tag="q2rp")
                    nc.vector.tensor_sub(rp[:, 0:1], t1[:], hsl[:, 1000:1001])
                    nc.vector.tensor_sub(rp[:, 1:2], t1[:], hsl[:, 0:1])
                    nc.vector.tensor_sub(rp[:, 2:3], t2[:], hsl[:, 1:2])
                    indap = (cs["ind_h"][0:125, t * 15:t * 15 + 15] if t < 11
                             else cs["ind_hL"][:])
                    psq_args.append((indap, rp))

                if b + 1 < BC:
                    xnext = load_x(b + 1)   # prefetch during conv2T

                # ---- conv2 transposed: h2T[wchunk][w, (t,co,hr)] ----
                h2ts = []
                for (w0, M) in LKS:
                    h2T = h2tp.tile([128, 1280], f16, tag=f"h2T{w0}")
                    for q in range(3):
                        ps = c2ps.tile([128, 512], f32, tag="c2ps")
                        for j in range(4):
                            t = 4 * q + j
                            if t < 11:
                                for dw in range(3):
                                    nc.tensor.matmul(
                                        ps[0:M, j * 115:j * 115 + 115],
                                        h1t[:, t * W1W + w0 + dw:
                                            t * W1W + w0 + dw + M],
                                        cs["w2bt"][:, dw * 115:dw * 115 + 115],
                                        start=(dw == 0), stop=(dw == 2),
                                        skip_group_check=True)
                            else:
                                for dw in range(3):
                                    nc.tensor.matmul(
                                        ps[0:M, 345:360],
                                        h1L[:, w0 + dw:w0 + dw + M],
                                        cs["w2btL"][:, dw * 15:dw * 15 + 15],
                                        start=(dw == 0), stop=(dw == 2),
                                        skip_group_check=True)
                        ncols = 460 if q < 2 else 360
                        if q < 2:
                            nc.scalar.activation(
                                h2T[0:M, q * 460:q * 460 + ncols],
                                ps[0:M, 0:ncols], AF.Relu)
                        else:
                            nc.vector.tensor_scalar_max(
                                h2T[0:M, q * 460:q * 460 + ncols],
                                ps[0:M, 0:ncols], 0.0)
                    h2ts.append(h2T)

                # ---- deferred q2 reduction + DMA ----
                psq2 = qps.tile([15, 3], f32, tag="psq")
                for ti, (indap, rp) in enumerate(psq_args):
                    nc.tensor.matmul(psq2[:], indap, rp[:],
                                     start=(ti == 0), stop=(ti == 11))
                q2sb = wp.tile([15, 3], f32, tag="q2sb")
                nc.vector.tensor_copy(q2sb[:], psq2[:])
                nc.sync.dma_start(q2sP[b], q2sb[:])

                # ---- linear: out[10, (t,co,hr)]; per-chunk out DMA ----
                outsb = outp.tile([10, 1280], f32, tag="outsb")
                for (n0, Nc) in NCCH:
                    pl = lps.tile([10, 512], f32, tag="lps")
                    for k, (k0, K) in enumerate(LKS):
                        nc.tensor.matmul(pl[0:10, 0:Nc],
                                         cs["wlt"][0:K, k * 10:k * 10 + 10],
                                         h2ts[k][0:K, n0:n0 + Nc],
                                         start=(k == 0), stop=(k == 7),
                                         skip_group_check=True)
                    nc.vector.tensor_copy(outsb[:, n0:n0 + Nc],
                                          pl[0:10, 0:Nc])
                    nc.sync.dma_start(outP[b, :, n0:n0 + Nc],
                                      outsb[:, n0:n0 + Nc])
    nc.compile()
    return nc


_CACHE = {}
LAST_RES = None


def kernel(x, W1, b1, W2, a1, a2, b2, Wl, bl, **kw):
    x = np.asarray(x, np.float32)
    W1n = np.asarray(W1, np.float32)
    W2n = np.asarray(W2, np.float32)
    a1n = np.asarray(a1, np.float32)
    a2n = np.asarray(a2, np.float32)
    b1n = float(np.asarray(b1, np.float32))
    b2n = float(np.asarray(b2, np.float32))
    Wln = np.asarray(Wl, np.float32)
    bln = np.asarray(bl, np.float32)
    B = x.shape[0]

    # host: filter hashes + layer-1 query hash -> mask1
    fh1 = _filter_hash(W1n, a1n, b1n)
    fh2 = _filter_hash(W2n, a2n, b2n)
    q1v = np.empty((B, 27), np.float64)   # columns ci*9 + i*3 + j
    for i in range(3):
        for j in range(3):
            s = x[:, :, i:i + H1, j:j + W1W].sum(axis=(2, 3),
                                                 dtype=np.float64)
            for ci in range(3):
                q1v[:, ci * 9 + i * 3 + j] = s[:, ci]
    qh1 = _qhash(q1v, a1n, b1n)
    mask1 = (fh1[None, :] == qh1[:, None]).astype(np.float32)   # [B, 5]

    consts = _consts_full(W1n, W2n, Wln)
    cshapes = {k: (v.shape, _CDTYPES[k]) for k, v in consts.items()}
    cshapes["m1"] = ((125, BC), f32)
    cshapes["m1L"] = ((25, BC), f32)
    if "nc" not in _CACHE:
        _CACHE["nc"] = _build_nc(cshapes)
    nc = _CACHE["nc"]

    n_cores = 8
    xprep = np.zeros((B, 912, WX), np.float16)
    for t in range(11):
        xprep[:, t * 81:t * 81 + 81, :] = \
            x[:, :, 23 * t:23 * t + 27, :].reshape(B, 81, WX)
    xprep[:, 891:912, :] = x[:, :, 253:260, :].reshape(B, 21, WX)

    m1e = np.repeat(mask1, 25, axis=1).T.astype(np.float32)     # [125, B]
    m1Le = np.repeat(mask1, 5, axis=1).T.astype(np.float32)     # [25, B]

    in_maps = []
    for i in range(n_cores):
        m = {"x": np.ascontiguousarray(xprep[i * BC:(i + 1) * BC]),
             "m1": np.ascontiguousarray(m1e[:, i * BC:(i + 1) * BC]),
             "m1L": np.ascontiguousarray(m1Le[:, i * BC:(i + 1) * BC])}
        m.update(consts)
        in_maps.append(m)
    res = run_bass_kernel_spmd(nc, in_maps, core_ids=list(range(n_cores)),
                               **kw)
    global LAST_RES
    LAST_RES = res
    lin = np.concatenate([res.results[i]["out"] for i in range(n_cores)],
                         axis=0)                      # [B, 10, 1280]
    q2s = np.concatenate([res.results[i]["q2s"] for i in range(n_cores)],
                         axis=0)                      # [B, 15, 3]

    # host: finish layer-2 query hash -> mask2
    q2v = np.empty((B, 45), np.float64)
    for ci in range(5):
        for i in range(3):
            for j in range(3):
                q2v[:, ci * 9 + i * 3 + j] = q2s[:, ci * 3 + i, j]
    qh2 = _qhash(q2v, a2n, b2n)
    mask2 = (fh2[None, :] == qh2[:, None]).astype(np.float32)   # [B, 5]

    # unscramble columns (t,co,hr) -> (co,h), apply mask2 and bias
    colmap = np.empty(1280, np.int64)
    for t in range(11):
        for co in range(5):
            for hr in range(23):
                colmap[co * 256 + 23 * t + hr] = t * 115 + co * 23 + hr
    for co in range(5):
        for hr in range(3):
            colmap[co * 256 + 253 + hr] = 1265 + co * 3 + hr
    out = lin[:, :, colmap]                           # [B, 10, 1280]
    out = out.transpose(0, 2, 1).reshape(B, 5, H2, 10)
    out = out * mask2[:, :, None, None] + bln[None, None, None, :]
    return np.ascontiguousarray(out, np.float32)
